# revision 1
# baseline (speedup 1.0000x reference)
"""Hetero-GNN (3x GATv2) Trainium2 kernel.

Strategy (8 cores, full I/O):
  - dst-partition both node types across the 8 cores (6250 dst rows each).
  - Phase 1 (on device, replicated): hl_r = x_src @ Wl_r for each relation,
    stored in DRAM as fp32 rows [feat(128) | 1.0 | att.hl] (130 cols, 520B);
    hr_r for the core's own dst slice as [feat(128) | att.hr] (129 cols).
  - Phase 2: edges sorted by dst, dsts binned into 49 degree-balanced
    windows of 128 dst slots. Per 128-edge subchunk: indirect-DMA row
    gathers of hl[src] and hr[dst], z = g + h, leaky-relu via
    e = (att.g + att.h) + 0.8 * sum(att * relu(-z)), w = exp(e) (exact
    softmax without max-subtraction; logits are O(5) so exp is safe),
    one-hot weighted matrix S[k, d] = w_k * (slot_k == d) built with a
    single fused tensor_scalar, then TensorE matmul S^T @ [g | 1]
    accumulates numerator and denominator in PSUM over the window.
  - Window epilogue: out = relu(mean_r(acc / den)), written per dst slot;
    host inverts the window permutation and concatenates core slices.
No collectives needed: inputs replicated, outputs disjoint.
"""

import numpy as np
import ml_dtypes

import concourse.bass as bass
import concourse.tile as tile
from concourse import mybir
from concourse.bass_utils import run_bass_kernel_spmd

P = 128
NCORES = 8
N = 50000          # nodes per type
D = 128            # in feats
C = 128            # out feats
E = 600000         # edges per relation
ND = N // NCORES   # 6250 dst nodes per core
NW = 49            # windows per core (49*128 = 6272 >= 6250)
DSTPAD = NW * P    # 6272
NNP = 392 * P      # 50176 padded source-node count
HLW = 130          # hl row: 128 feats | 1.0 | att.hl
HRW = 129          # hr row: 128 feats | att.hr
SLOPE = 0.2
RELS = ("ab", "ba", "aa")
BF16 = mybir.dt.bfloat16
F32 = mybir.dt.float32
I32 = mybir.dt.int32

_BUILD_CACHE = {}


def _build_program(subs):
    """subs: dict rel -> subchunks-per-window (compile-time constants)."""
    nc = bass.Bass()

    # ---- I/O declarations ----
    inp = {}
    for nm, shape, dt in [
        ("xT_a", [P, NNP], BF16), ("xT_b", [P, NNP], BF16),
        ("xTd_a", [P, DSTPAD], BF16), ("xTd_b", [P, DSTPAD], BF16),
        ("iota", [P, P], F32),
    ]:
        inp[nm] = nc.dram_tensor(nm, shape, dt, kind="ExternalInput")
    for r in RELS:
        ns = NW * subs[r]
        for nm, shape, dt in [
            (f"wl_{r}", [P, HLW], BF16), (f"wr_{r}", [P, HRW], BF16),
            (f"att_{r}", [P, P], F32),
            (f"srcT_{r}", [P, ns], I32), (f"dstT_{r}", [P, ns], I32),
            (f"relT_{r}", [P, ns], F32),
        ]:
            inp[nm] = nc.dram_tensor(nm, shape, dt, kind="ExternalInput")

    out_a = nc.dram_tensor("out_a", [DSTPAD, C], F32, kind="ExternalOutput")
    out_b = nc.dram_tensor("out_b", [DSTPAD, C], F32, kind="ExternalOutput")

    hl = {r: nc.dram_tensor(f"hl_{r}", [NNP, HLW], F32) for r in RELS}
    hr = {r: nc.dram_tensor(f"hr_{r}", [DSTPAD, HRW], F32) for r in RELS}

    src_of = {"ab": "xT_a", "ba": "xT_b", "aa": "xT_a"}
    dst_of = {"ab": "xTd_b", "ba": "xTd_a", "aa": "xTd_a"}

    with tile.TileContext(nc) as tc:
        with (
            tc.tile_pool(name="consts", bufs=1) as consts,
            tc.tile_pool(name="xin", bufs=3) as xin,
            tc.tile_pool(name="p1ps", bufs=3, space="PSUM") as p1ps,
            tc.tile_pool(name="p1ep", bufs=3) as p1ep,
            tc.tile_pool(name="gath", bufs=3) as gath,
            tc.tile_pool(name="work", bufs=3) as work,
            tc.tile_pool(name="small", bufs=4) as small,
            tc.tile_pool(name="p2ps", bufs=4, space="PSUM") as p2ps,
            tc.tile_pool(name="outp", bufs=4) as outp,
        ):
            # ---- load constants ----
            iota_t = consts.tile([P, P], F32, tag="iota")
            nc.sync.dma_start(out=iota_t[:], in_=inp["iota"][:])
            wl_t, wr_t, att_t, srcT_t, dstT_t, relT_t = {}, {}, {}, {}, {}, {}
            for r in RELS:
                ns = NW * subs[r]
                wl_t[r] = consts.tile([P, HLW], BF16, tag=f"wl{r}", name=f"wl{r}")
                wr_t[r] = consts.tile([P, HRW], BF16, tag=f"wr{r}", name=f"wr{r}")
                att_t[r] = consts.tile([P, P], F32, tag=f"att{r}", name=f"att{r}")
                srcT_t[r] = consts.tile([P, ns], I32, tag=f"src{r}", name=f"src{r}")
                dstT_t[r] = consts.tile([P, ns], I32, tag=f"dst{r}", name=f"dst{r}")
                relT_t[r] = consts.tile([P, ns], F32, tag=f"rel{r}", name=f"rel{r}")
                for t, nm in [
                    (wl_t[r], f"wl_{r}"), (wr_t[r], f"wr_{r}"),
                    (att_t[r], f"att_{r}"), (srcT_t[r], f"srcT_{r}"),
                    (dstT_t[r], f"dstT_{r}"), (relT_t[r], f"relT_{r}"),
                ]:
                    nc.sync.dma_start(out=t[:], in_=inp[nm][:])
            xd_t = {}
            for nm in ("xTd_a", "xTd_b"):
                xd_t[nm] = consts.tile([P, DSTPAD], BF16, tag=nm, name=nm)
                nc.sync.dma_start(out=xd_t[nm][:], in_=inp[nm][:])

            # ---- phase 1: projections ----
            def emit_phase1(r):
                xsrc = inp[src_of[r]]
                # hl: 49 outer chunks x 8 subchunks of 128 nodes
                for j in range(NNP // 1024):
                    xt = xin.tile([P, 1024], BF16, tag="xchunk")
                    nc.gpsimd.dma_start(
                        out=xt[:], in_=xsrc[:, j * 1024:(j + 1) * 1024])
                    ep = p1ep.tile([P, 8 * HLW], F32, tag="hl_ep")
                    ep3 = ep[:].rearrange("p (s c) -> p s c", c=HLW)
                    for s in range(8):
                        ps = p1ps.tile([P, HLW], F32, tag="p1ps")
                        nc.tensor.matmul(
                            out=ps[:], lhsT=xt[:, s * P:(s + 1) * P],
                            rhs=wl_t[r][:], start=True, stop=True)
                        nc.scalar.copy(out=ep3[:, s, :], in_=ps[:])
                    nc.vector.memset(ep3[:, :, 128:129], 1.0)
                    nc.scalar.dma_start(
                        out=hl[r][j * 1024:(j + 1) * 1024, :].rearrange(
                            "(s p) c -> p s c", p=P),
                        in_=ep3[:, :, :])
                # hr: 49 chunks of 128 dst rows, batches of 8
                xd = xd_t[dst_of[r]]
                for g in range((NW + 7) // 8):
                    cnt = min(8, NW - g * 8)
                    ep = p1ep.tile([P, 8 * HRW], F32, tag="hr_ep")
                    ep3 = ep[:].rearrange("p (s c) -> p s c", c=HRW)
                    for s in range(cnt):
                        jj = g * 8 + s
                        ps = p1ps.tile([P, HLW], F32, tag="p1ps", name="hr_ps")[:, :HRW]
                        nc.tensor.matmul(
                            out=ps[:], lhsT=xd[:, jj * P:(jj + 1) * P],
                            rhs=wr_t[r][:], start=True, stop=True)
                        nc.scalar.copy(out=ep3[:, s, :], in_=ps[:])
                    nc.scalar.dma_start(
                        out=hr[r][g * 1024:g * 1024 + cnt * P, :].rearrange(
                            "(s p) c -> p s c", p=P),
                        in_=ep3[:, :cnt, :])

            for r in RELS:
                emit_phase1(r)

            # ---- phase 2: edge processing, window-major ----
            def emit_window_rel(r, w):
                SUB = subs[r]
                i0 = w * SUB
                # gathers
                gt = gath.tile([P, SUB * HLW], F32, tag="G")
                ht = gath.tile([P, SUB * HRW], F32, tag="H")
                for s in range(SUB):
                    nc.gpsimd.indirect_dma_start(
                        out=gt[:, s * HLW:(s + 1) * HLW], out_offset=None,
                        in_=hl[r][:],
                        in_offset=bass.IndirectOffsetOnAxis(
                            ap=srcT_t[r][:, i0 + s:i0 + s + 1], axis=0))
                    nc.gpsimd.indirect_dma_start(
                        out=ht[:, s * HRW:(s + 1) * HRW], out_offset=None,
                        in_=hr[r][:],
                        in_offset=bass.IndirectOffsetOnAxis(
                            ap=dstT_t[r][:, i0 + s:i0 + s + 1], axis=0))
                g3 = gt[:].rearrange("p (s c) -> p s c", c=HLW)
                h3 = ht[:].rearrange("p (s c) -> p s c", c=HRW)
                # z = g + h (feat cols), sdot = att.g + att.h
                zt = work.tile([P, SUB * P], F32, tag="z")
                z3 = zt[:].rearrange("p (s c) -> p s c", c=P)
                nc.vector.tensor_tensor(
                    out=z3[:, :, :], in0=g3[:, :, 0:P], in1=h3[:, :, 0:P],
                    op=mybir.AluOpType.add)
                sdot = small.tile([P, SUB], F32, tag="sdot")
                nc.vector.tensor_tensor(
                    out=sdot[:].rearrange("p (s c) -> p s c", c=1),
                    in0=g3[:, :, 129:130], in1=h3[:, :, 128:129],
                    op=mybir.AluOpType.add)
                # r = relu(-z)
                rt = work.tile([P, SUB * P], F32, tag="rneg")
                nc.scalar.activation(
                    out=rt[:], in_=zt[:],
                    func=mybir.ActivationFunctionType.Relu, scale=-1.0)
                # value-path bf16 copy of [feat | 1] cols
                gb = work.tile([P, SUB * HRW], BF16, tag="gb16")
                nc.scalar.copy(
                    out=gb[:].rearrange("p (s c) -> p s c", c=HRW),
                    in_=g3[:, :, 0:HRW])
                # racc[s] = sum(att * r) per subchunk
                racc = small.tile([P, SUB], F32, tag="racc")
                for s in range(SUB):
                    ttrd = work.tile([P, P], F32, tag="ttrd", name="ttrd")
                    nc.vector.tensor_tensor(
                        out=ttrd[:], in0=rt[:, s * P:(s + 1) * P],
                        in1=att_t[r][:], op=mybir.AluOpType.mult)
                    nc.vector.tensor_reduce(
                        out=racc[:, s:s + 1], in_=ttrd[:],
                        axis=mybir.AxisListType.X, op=mybir.AluOpType.add)
                # e = sdot - 0.8 * racc ; w = exp(e)
                et = small.tile([P, SUB], F32, tag="e")
                nc.vector.tensor_scalar(
                    out=et[:], in0=racc[:], scalar1=(1.0 - SLOPE),
                    scalar2=None, op0=mybir.AluOpType.mult)
                nc.vector.tensor_tensor(
                    out=et[:], in0=et[:], in1=sdot[:],
                    op=mybir.AluOpType.add)
                wt = small.tile([P, SUB], F32, tag="w")
                nc.scalar.activation(
                    out=wt[:], in_=et[:],
                    func=mybir.ActivationFunctionType.Exp)
                # S[k, d] = w_k * (slot_k == d); matmul accumulate
                st = work.tile([P, SUB * P], BF16, tag="S")
                ps = p2ps.tile([P, HRW], F32, tag="acc")
                for s in range(SUB):
                    nc.vector.tensor_scalar(
                        out=st[:, s * P:(s + 1) * P], in0=iota_t[:],
                        scalar1=relT_t[r][:, i0 + s:i0 + s + 1],
                        scalar2=wt[:, s:s + 1],
                        op0=mybir.AluOpType.is_equal,
                        op1=mybir.AluOpType.mult)
                    nc.tensor.matmul(
                        out=ps[:], lhsT=st[:, s * P:(s + 1) * P],
                        rhs=gb[:, s * HRW:(s + 1) * HRW],
                        start=(s == 0), stop=(s == SUB - 1))
                # normalize: o = acc / (den + eps)
                den = small.tile([P, 1], F32, tag="den")
                nc.vector.tensor_scalar(
                    out=den[:], in0=ps[:, 128:129], scalar1=1e-12,
                    scalar2=None, op0=mybir.AluOpType.add)
                rcp = small.tile([P, 1], F32, tag="rcp")
                nc.vector.reciprocal(out=rcp[:], in_=den[:])
                ot = outp.tile([P, P], F32, tag=f"o_{r}")
                nc.vector.tensor_scalar(
                    out=ot[:], in0=ps[:, 0:P], scalar1=rcp[:],
                    scalar2=None, op0=mybir.AluOpType.mult)
                return ot

            for w in range(NW):
                # relation ab -> out_b
                o_ab = emit_window_rel("ab", w)
                ob = outp.tile([P, P], F32, tag="outb")
                nc.scalar.activation(
                    out=ob[:], in_=o_ab[:],
                    func=mybir.ActivationFunctionType.Relu)
                nc.sync.dma_start(
                    out=out_b[w * P:(w + 1) * P, :], in_=ob[:])
                # relations ba, aa -> out_a
                o_ba = emit_window_rel("ba", w)
                o_aa = emit_window_rel("aa", w)
                nc.vector.tensor_tensor(
                    out=o_ba[:], in0=o_ba[:], in1=o_aa[:],
                    op=mybir.AluOpType.add)
                oa = outp.tile([P, P], F32, tag="outa")
                nc.scalar.activation(
                    out=oa[:], in_=o_ba[:],
                    func=mybir.ActivationFunctionType.Relu, scale=0.5)
                nc.sync.dma_start(
                    out=out_a[w * P:(w + 1) * P, :], in_=oa[:])

    _spill_dma_waits(nc)
    return nc


def _spill_dma_waits(nc):
    """The bundled walrus build only accepts one embedded sync-wait per DMA
    pseudo-instruction. Move multi-waits onto a NoOp on the issuing engine
    (engines decode in order, so the DMA stays gated)."""
    for bbb in nc.bb_map.values():
        insts = bbb.bb.instructions
        out = []
        for ins in insts:
            si = getattr(ins, "sync_info", None)
            ow = list(si.on_wait) if si is not None and si.on_wait else []
            if len(ow) >= 2:
                for w in ow:
                    nop = mybir.InstNoOp(
                        name=nc.get_next_instruction_name(), ins=[], outs=[],
                        engine=ins.engine)
                    nop.sync_info = mybir.SyncInfo(on_wait=[w], on_update=[])
                    out.append(nop)
                ins.sync_info = mybir.SyncInfo(
                    on_wait=[], on_update=list(si.on_update or []))
            out.append(ins)
        insts[:] = out


# ---------------- host-side preprocessing ----------------

def _balanced_windows(deg):
    """Assign ND dsts to NW bins of <=128 slots, balancing total degree.
    Serpentine assignment over degree-sorted dsts. Returns win[d], slot[d]."""
    order = np.argsort(-deg, kind="stable")
    win = np.empty(ND, np.int32)
    slot = np.empty(ND, np.int32)
    fill = np.zeros(NW, np.int32)
    b = 0
    direction = 1
    for i, d in enumerate(order):
        # serpentine over bins, skipping full ones
        tries = 0
        while fill[b] >= P:
            b += direction
            if b == NW or b < 0:
                direction = -direction
                b += direction
            tries += 1
            assert tries <= 2 * NW
        win[d] = b
        slot[d] = fill[b]
        fill[b] += 1
        b += direction
        if b == NW or b < 0:
            direction = -direction
            b += direction
    return win, slot


def _pack_edges(src, dst_local, win, slot, sub):
    """Group edges by window, pad each window to sub*128 slots.
    Returns srcT, dstT, relT transposed [128, NW*sub] arrays."""
    ewin = win[dst_local]
    order = np.argsort(ewin, kind="stable")
    src_s = src[order]
    dstl_s = dst_local[order]
    erel_s = slot[dst_local][order].astype(np.float32)
    ewin_s = ewin[order]
    counts = np.bincount(ewin_s, minlength=NW)
    offs = np.zeros(NW + 1, np.int64)
    np.cumsum(counts, out=offs[1:])
    pos = np.arange(len(src_s)) - offs[ewin_s]
    flat = ewin_s.astype(np.int64) * (sub * P) + pos
    nslots = NW * sub * P
    srcp = np.zeros(nslots, np.int32)
    dstp = np.zeros(nslots, np.int32)
    relp = np.full(nslots, 999.0, np.float32)
    srcp[flat] = src_s
    dstp[flat] = dstl_s
    relp[flat] = erel_s
    to_T = lambda a: np.ascontiguousarray(a.reshape(NW * sub, P).T)
    return to_T(srcp), to_T(dstp), to_T(relp)


def kernel(**inputs):
    x_a = np.asarray(inputs["x_a"], np.float32)
    x_b = np.asarray(inputs["x_b"], np.float32)
    edges = {r: np.asarray(inputs[f"edge_{r}"]).astype(np.int64) for r in RELS}

    # shared device inputs
    def padT(x, cols):
        out = np.zeros((P, cols), ml_dtypes.bfloat16)
        out[:, :x.shape[0]] = x.T.astype(ml_dtypes.bfloat16)
        return out

    shared = {
        "xT_a": padT(x_a, NNP),
        "xT_b": padT(x_b, NNP),
        "iota": np.broadcast_to(
            np.arange(P, dtype=np.float32), (P, P)).copy(),
    }
    for r in RELS:
        Wl = np.asarray(inputs[f"Wl_{r}"], np.float32)
        Wr = np.asarray(inputs[f"Wr_{r}"], np.float32)
        att = np.asarray(inputs[f"att_{r}"], np.float32)
        for nm in ("bl", "br", "bias"):
            assert not np.any(np.asarray(inputs[f"{nm}_{r}"])), \
                f"nonzero {nm}_{r} not supported"
        wl = np.zeros((P, HLW), np.float32)
        wl[:, :C] = Wl
        wl[:, 129] = Wl @ att
        wr = np.zeros((P, HRW), np.float32)
        wr[:, :C] = Wr
        wr[:, 128] = Wr @ att
        shared[f"wl_{r}"] = wl.astype(ml_dtypes.bfloat16)
        shared[f"wr_{r}"] = wr.astype(ml_dtypes.bfloat16)
        shared[f"att_{r}"] = np.broadcast_to(att, (P, P)).copy()

    # per-core graph structure
    dst_type = {"ab": "b", "ba": "a", "aa": "a"}
    # sort edges by dst once per relation
    sorted_e = {}
    for r in RELS:
        s, d = edges[r][0], edges[r][1]
        o = np.argsort(d, kind="stable")
        sorted_e[r] = (s[o].astype(np.int32), d[o].astype(np.int32))

    core_data = []
    for c in range(NCORES):
        base = c * ND
        # combined degree per dst type for window balance
        deg = {"a": np.zeros(ND, np.int64), "b": np.zeros(ND, np.int64)}
        loc = {}
        for r in RELS:
            s, d = sorted_e[r]
            lo, hi = np.searchsorted(d, [base, base + ND])
            dl = (d[lo:hi] - base).astype(np.int64)
            loc[r] = (s[lo:hi], dl)
            deg[dst_type[r]] += np.bincount(dl, minlength=ND)
        winslot = {t: _balanced_windows(deg[t]) for t in ("a", "b")}
        core_data.append((loc, winslot))

    # global SUB per relation
    subs = {}
    for r in RELS:
        mx = 0
        for loc, winslot in core_data:
            win, _ = winslot[dst_type[r]]
            s, dl = loc[r]
            wc = np.bincount(win[dl], minlength=NW)
            mx = max(mx, int(wc.max()))
        subs[r] = max(1, -(-mx // P))

    key = tuple(sorted(subs.items()))
    if key not in _BUILD_CACHE:
        _BUILD_CACHE[key] = _build_program(subs)
    nc = _BUILD_CACHE[key]

    in_maps = []
    for c in range(NCORES):
        base = c * ND
        loc, winslot = core_data[c]
        m = dict(shared)

        def dslice(x):
            sl = np.zeros((DSTPAD, D), np.float32)
            end = min(N, base + DSTPAD)
            sl[:end - base] = x[base:end]
            return np.ascontiguousarray(sl.T).astype(ml_dtypes.bfloat16)

        m["xTd_a"] = dslice(x_a)
        m["xTd_b"] = dslice(x_b)
        for r in RELS:
            win, slot = winslot[dst_type[r]]
            s, dl = loc[r]
            srcT, dstT, relT = _pack_edges(s, dl, win, slot, subs[r])
            m[f"srcT_{r}"] = srcT
            m[f"dstT_{r}"] = dstT
            m[f"relT_{r}"] = relT
        in_maps.append(m)

    res = run_bass_kernel_spmd(nc, in_maps, core_ids=list(range(NCORES)))

    out_a = np.empty((N, C), np.float32)
    out_b = np.empty((N, C), np.float32)
    for c in range(NCORES):
        base = c * ND
        _, winslot = core_data[c]
        for t, full in (("a", out_a), ("b", out_b)):
            win, slot = winslot[t]
            rowmap = win.astype(np.int64) * P + slot
            dev = res.results[c][f"out_{t}"]
            full[base:base + ND] = dev[rowmap]
    return out_a, out_b



# revision 5
# speedup vs baseline: 6.8668x; 6.8668x over previous
"""Hetero-GNN (3x GATv2) Trainium2 kernel — transfer-optimized.

Strategy (8 cores, full I/O):
  - Each core owns a 6250-node dst slice of both node types. Host assigns
    dsts to 49 degree-balanced windows of 128 slots; the core's x slice is
    uploaded window-permuted and feature-major as ONE bf16 tensor
    (xloc [128, 12544] = [type a | type b]).
  - On device an 8-core AllGather reconstructs the full node features
    (x_full [1024, 12544], rank-major blocks), so the big x tensors are
    uploaded exactly once instead of replicated 8x.
  - Edge structure per relation is uploaded compact: src index (uint16,
    window-position space, remapped host-side) + dst slot-in-window
    (uint8); widened on device. Per-edge dst row = 128*w + slot is
    computed on device, so no dst index upload.
  - Phase 1 (replicated): hl_r = x_src @ Wl_r for all 50176 node rows
    as fp32 [feat(128) | 1.0 | att.hl] (130 cols); hr_r for the core's
    own window-ordered dst slice as [feat(128) | att.hr] (129 cols).
  - Phase 2 (as before): per 128-edge subchunk indirect-DMA row gathers
    of hl[src] and hr[dst], e = (att.g + att.h) + (1-slope)*sum(att*
    relu(-z)), w = exp(e), one-hot weighted S built with a fused
    tensor_scalar, TensorE matmul S^T @ [g | 1] accumulates numerator
    and denominator in PSUM per window.
  - Epilogue: out = relu(mean_r(acc / den)) written bf16 into a single
    merged output [12544, 128] (a rows | b rows); host inverts the
    window permutation.
  - run_bass_via_pjrt is wrapped with a caching version (same semantics)
    so the jitted executable and zero output buffers are reused across
    calls instead of re-traced + re-compiled every invocation.
"""

import numpy as np
import ml_dtypes

import jax
from jax.sharding import Mesh, PartitionSpec, NamedSharding

from jax.experimental.shard_map import shard_map

import concourse.bass as bass
import concourse.tile as tile
from concourse import mybir
import concourse.bass2jax as _b2j
from concourse.bass_utils import run_bass_kernel_spmd, BassKernelResults

P = 128
NCORES = 8
N = 50000          # nodes per type
D = 128            # in feats
C = 128            # out feats
E = 600000         # edges per relation
ND = N // NCORES   # 6250 dst nodes per core
NW = 49            # windows per core (49*128 = 6272 >= 6250)
DSTPAD = NW * P    # 6272
NNP = NCORES * DSTPAD  # 50176 gathered node rows (8 rank blocks)
XW = 2 * DSTPAD    # 12544 xloc cols: [a | b]
HLW = 130          # hl row: 128 feats | 1.0 | att.hl
HRW = 129          # hr row: 128 feats | att.hr
HRPAD = 256        # scratch rows after hr (absorbs pad-slot dst idx)
SLOPE = 0.2
RELS = ("ab", "ba", "aa")
SRC_TYPE = {"ab": "a", "ba": "b", "aa": "a"}
DST_TYPE = {"ab": "b", "ba": "a", "aa": "a"}
TYPE_OFF = {"a": 0, "b": DSTPAD}
BF16 = mybir.dt.bfloat16
F32 = mybir.dt.float32
I32 = mybir.dt.int32
U16 = mybir.dt.uint16
U8 = mybir.dt.uint8

_BUILD_CACHE = {}


def _build_program(subs):
    """subs: dict rel -> subchunks-per-window (compile-time constants)."""
    nc = bass.Bass(num_devices=NCORES)
    nc._hgnn_fast = True

    ns = {r: NW * subs[r] for r in RELS}
    ns_all = sum(ns.values())
    ecol0 = {}
    off = 0
    for r in RELS:
        ecol0[r] = off
        off += ns[r]

    # ---- I/O ----
    xloc = nc.dram_tensor("xloc", [P, XW], BF16, kind="ExternalInput")
    srcu = nc.dram_tensor("srcu", [P, ns_all], U16, kind="ExternalInput")
    relu8 = nc.dram_tensor("relu8", [P, ns_all], U8, kind="ExternalInput")
    wlr = nc.dram_tensor("wlr", [P, 3 * (HLW + HRW)], BF16, kind="ExternalInput")
    attr = nc.dram_tensor("attr", [1, 3 * P], F32, kind="ExternalInput")
    out = nc.dram_tensor("out", [XW, C], BF16, kind="ExternalOutput")

    bounce = nc.dram_tensor("bounce", [P, XW], BF16)
    x_full = nc.dram_tensor("x_full", [NCORES * P, XW], BF16, addr_space="Shared")
    hl = {r: nc.dram_tensor(f"hl_{r}", [NNP, HLW], F32) for r in RELS}
    hr = {r: nc.dram_tensor(f"hr_{r}", [DSTPAD + HRPAD, HRW], F32) for r in RELS}

    with tile.TileContext(nc) as tc:
        with (
            tc.tile_pool(name="consts", bufs=1) as consts,
            tc.tile_pool(name="xin", bufs=3) as xin,
            tc.tile_pool(name="p1ps", bufs=3, space="PSUM") as p1ps,
            tc.tile_pool(name="p1ep", bufs=3) as p1ep,
            tc.tile_pool(name="gath", bufs=3) as gath,
            tc.tile_pool(name="work", bufs=3) as work,
            tc.tile_pool(name="small", bufs=4) as small,
            tc.tile_pool(name="p2ps", bufs=4, space="PSUM") as p2ps,
            tc.tile_pool(name="outp", bufs=4) as outp,
        ):
            # ---- AllGather x ----
            nc.sync.dma_start(out=bounce[:], in_=xloc[:])
            nc.gpsimd.collective_compute(
                "AllGather", mybir.AluOpType.bypass,
                replica_groups=[list(range(NCORES))],
                ins=[bounce[:].opt()], outs=[x_full[:].opt()])

            # ---- constants / widening ----
            iota_i = consts.tile([P, P], I32, tag="iota_i")
            nc.gpsimd.iota(iota_i[:], pattern=[[1, P]], base=0,
                           channel_multiplier=0)
            iota_t = consts.tile([P, P], F32, tag="iota")
            nc.vector.tensor_scalar(
                out=iota_t[:], in0=iota_i[:], scalar1=0, scalar2=None,
                op0=mybir.AluOpType.add)

            wlr_t = consts.tile([P, 3 * (HLW + HRW)], BF16, tag="wlr")
            nc.sync.dma_start(out=wlr_t[:], in_=wlr[:])
            wl_t = {r: wlr_t[:, i * (HLW + HRW):i * (HLW + HRW) + HLW]
                    for i, r in enumerate(RELS)}
            wr_t = {r: wlr_t[:, i * (HLW + HRW) + HLW:(i + 1) * (HLW + HRW)]
                    for i, r in enumerate(RELS)}

            att_sb = consts.tile([1, 3 * P], F32, tag="attr")
            nc.sync.dma_start(out=att_sb[:], in_=attr[:])
            ones1 = consts.tile([1, P], F32, tag="ones1")
            nc.vector.memset(ones1[:], 1.0)
            att_t = {}
            for i, r in enumerate(RELS):
                ps = p1ps.tile([P, HLW], F32, tag="p1ps", name=f"attb{r}")
                nc.tensor.matmul(
                    out=ps[:, 0:P], lhsT=ones1[:],
                    rhs=att_sb[:, i * P:(i + 1) * P], start=True, stop=True)
                att_t[r] = consts.tile([P, P], F32, tag=f"att{r}", name=f"att{r}")
                nc.scalar.copy(out=att_t[r][:], in_=ps[:, 0:P])

            srcu_t = consts.tile([P, ns_all], U16, tag="srcu")
            nc.sync.dma_start(out=srcu_t[:], in_=srcu[:])
            src_t = consts.tile([P, ns_all], I32, tag="srci")
            nc.vector.tensor_scalar(
                out=src_t[:], in0=srcu_t[:], scalar1=0, scalar2=None,
                op0=mybir.AluOpType.add)
            relu_t = consts.tile([P, ns_all], U8, tag="relu8")
            nc.sync.dma_start(out=relu_t[:], in_=relu8[:])
            relf_t = consts.tile([P, ns_all], F32, tag="relf")
            nc.vector.tensor_scalar(
                out=relf_t[:], in0=relu_t[:], scalar1=0, scalar2=None,
                op0=mybir.AluOpType.add)
            reli_t = consts.tile([P, ns_all], I32, tag="reli")
            nc.vector.tensor_scalar(
                out=reli_t[:], in0=relu_t[:], scalar1=0, scalar2=None,
                op0=mybir.AluOpType.add)

            srcT = {r: src_t[:, ecol0[r]:ecol0[r] + ns[r]] for r in RELS}
            relT = {r: relf_t[:, ecol0[r]:ecol0[r] + ns[r]] for r in RELS}
            relI = {r: reli_t[:, ecol0[r]:ecol0[r] + ns[r]] for r in RELS}

            # ---- phase 1 ----
            CH = 7 * P  # 896-col chunks; 6272 = 7 * 896

            def emit_phase1(r):
                to = TYPE_OFF[SRC_TYPE[r]]
                # hl over 8 rank blocks x 7 chunks of 896 node cols
                for rk in range(NCORES):
                    for j in range(7):
                        xt = xin.tile([P, CH], BF16, tag="xchunk")
                        nc.gpsimd.dma_start(
                            out=xt[:],
                            in_=x_full[rk * P:(rk + 1) * P,
                                       to + j * CH:to + (j + 1) * CH])
                        ep = p1ep.tile([P, 7 * HLW], F32, tag="hl_ep")
                        ep3 = ep[:].rearrange("p (s c) -> p s c", c=HLW)
                        for s in range(7):
                            ps = p1ps.tile([P, HLW], F32, tag="p1ps")
                            nc.tensor.matmul(
                                out=ps[:], lhsT=xt[:, s * P:(s + 1) * P],
                                rhs=wl_t[r], start=True, stop=True)
                            nc.scalar.copy(out=ep3[:, s, :], in_=ps[:])
                        nc.vector.memset(ep3[:, :, 128:129], 1.0)
                        base = rk * DSTPAD + j * CH
                        nc.scalar.dma_start(
                            out=hl[r][base:base + CH, :].rearrange(
                                "(s p) c -> p s c", p=P),
                            in_=ep3[:, :, :])
                # hr: this core's own window-ordered dst slice
                td = TYPE_OFF[DST_TYPE[r]]
                for g in range((NW + 7) // 8):
                    cnt = min(8, NW - g * 8)
                    xd = xin.tile([P, 8 * P], BF16, tag="xdchunk")
                    nc.gpsimd.dma_start(
                        out=xd[:, :cnt * P],
                        in_=xloc[:, td + g * 8 * P:td + (g * 8 + cnt) * P])
                    ep = p1ep.tile([P, 8 * HRW], F32, tag="hr_ep")
                    ep3 = ep[:].rearrange("p (s c) -> p s c", c=HRW)
                    for s in range(cnt):
                        ps = p1ps.tile([P, HLW], F32, tag="p1ps",
                                       name="hr_ps")[:, :HRW]
                        nc.tensor.matmul(
                            out=ps[:], lhsT=xd[:, s * P:(s + 1) * P],
                            rhs=wr_t[r], start=True, stop=True)
                        nc.scalar.copy(out=ep3[:, s, :], in_=ps[:])
                    nc.scalar.dma_start(
                        out=hr[r][g * 1024:g * 1024 + cnt * P, :].rearrange(
                            "(s p) c -> p s c", p=P),
                        in_=ep3[:, :cnt, :])
                # zero the pad region (absorbs pad-slot dst indices)
                zt = p1ep.tile([P, (HRPAD // P) * HRW], F32, tag="hr_zero")
                nc.vector.memset(zt[:], 0.0)
                nc.scalar.dma_start(
                    out=hr[r][DSTPAD:DSTPAD + HRPAD, :].rearrange(
                        "(s p) c -> p s c", p=P),
                    in_=zt[:].rearrange("p (s c) -> p s c", c=HRW))

            for r in RELS:
                emit_phase1(r)

            # ---- phase 2 ----
            def emit_window_rel(r, w):
                SUB = subs[r]
                i0 = w * SUB
                # per-edge dst row = 128*w + slot
                dsti = small.tile([P, SUB], I32, tag="dsti")
                nc.vector.tensor_scalar(
                    out=dsti[:], in0=relI[r][:, i0:i0 + SUB],
                    scalar1=P * w, scalar2=None, op0=mybir.AluOpType.add)
                # gathers
                gt = gath.tile([P, SUB * HLW], F32, tag="G")
                ht = gath.tile([P, SUB * HRW], F32, tag="H")
                for s in range(SUB):
                    nc.gpsimd.indirect_dma_start(
                        out=gt[:, s * HLW:(s + 1) * HLW], out_offset=None,
                        in_=hl[r][:],
                        in_offset=bass.IndirectOffsetOnAxis(
                            ap=srcT[r][:, i0 + s:i0 + s + 1], axis=0))
                    nc.gpsimd.indirect_dma_start(
                        out=ht[:, s * HRW:(s + 1) * HRW], out_offset=None,
                        in_=hr[r][:],
                        in_offset=bass.IndirectOffsetOnAxis(
                            ap=dsti[:, s:s + 1], axis=0))
                g3 = gt[:].rearrange("p (s c) -> p s c", c=HLW)
                h3 = ht[:].rearrange("p (s c) -> p s c", c=HRW)
                # z = g + h (feat cols), sdot = att.g + att.h
                zt = work.tile([P, SUB * P], F32, tag="z")
                z3 = zt[:].rearrange("p (s c) -> p s c", c=P)
                nc.vector.tensor_tensor(
                    out=z3[:, :, :], in0=g3[:, :, 0:P], in1=h3[:, :, 0:P],
                    op=mybir.AluOpType.add)
                sdot = small.tile([P, SUB], F32, tag="sdot")
                nc.vector.tensor_tensor(
                    out=sdot[:].rearrange("p (s c) -> p s c", c=1),
                    in0=g3[:, :, 129:130], in1=h3[:, :, 128:129],
                    op=mybir.AluOpType.add)
                # r = relu(-z)
                rt = work.tile([P, SUB * P], F32, tag="rneg")
                nc.scalar.activation(
                    out=rt[:], in_=zt[:],
                    func=mybir.ActivationFunctionType.Relu, scale=-1.0)
                # value-path bf16 copy of [feat | 1] cols
                gb = work.tile([P, SUB * HRW], BF16, tag="gb16")
                nc.scalar.copy(
                    out=gb[:].rearrange("p (s c) -> p s c", c=HRW),
                    in_=g3[:, :, 0:HRW])
                # racc[s] = sum(att * r) per subchunk
                racc = small.tile([P, SUB], F32, tag="racc")
                for s in range(SUB):
                    ttrd = work.tile([P, P], F32, tag="ttrd", name="ttrd")
                    nc.vector.tensor_tensor(
                        out=ttrd[:], in0=rt[:, s * P:(s + 1) * P],
                        in1=att_t[r][:], op=mybir.AluOpType.mult)
                    nc.vector.tensor_reduce(
                        out=racc[:, s:s + 1], in_=ttrd[:],
                        axis=mybir.AxisListType.X, op=mybir.AluOpType.add)
                # e = sdot - 0.8 * racc ; w = exp(e)
                et = small.tile([P, SUB], F32, tag="e")
                nc.vector.tensor_scalar(
                    out=et[:], in0=racc[:], scalar1=(1.0 - SLOPE),
                    scalar2=None, op0=mybir.AluOpType.mult)
                nc.vector.tensor_tensor(
                    out=et[:], in0=et[:], in1=sdot[:],
                    op=mybir.AluOpType.add)
                wt = small.tile([P, SUB], F32, tag="w")
                nc.scalar.activation(
                    out=wt[:], in_=et[:],
                    func=mybir.ActivationFunctionType.Exp)
                # S[k, d] = w_k * (slot_k == d); matmul accumulate
                st = work.tile([P, SUB * P], BF16, tag="S")
                ps = p2ps.tile([P, HRW], F32, tag="acc")
                for s in range(SUB):
                    nc.vector.tensor_scalar(
                        out=st[:, s * P:(s + 1) * P], in0=iota_t[:],
                        scalar1=relT[r][:, i0 + s:i0 + s + 1],
                        scalar2=wt[:, s:s + 1],
                        op0=mybir.AluOpType.is_equal,
                        op1=mybir.AluOpType.mult)
                    nc.tensor.matmul(
                        out=ps[:], lhsT=st[:, s * P:(s + 1) * P],
                        rhs=gb[:, s * HRW:(s + 1) * HRW],
                        start=(s == 0), stop=(s == SUB - 1))
                # normalize: o = acc / (den + eps)
                den = small.tile([P, 1], F32, tag="den")
                nc.vector.tensor_scalar(
                    out=den[:], in0=ps[:, 128:129], scalar1=1e-12,
                    scalar2=None, op0=mybir.AluOpType.add)
                rcp = small.tile([P, 1], F32, tag="rcp")
                nc.vector.reciprocal(out=rcp[:], in_=den[:])
                ot = outp.tile([P, P], F32, tag=f"o_{r}")
                nc.vector.tensor_scalar(
                    out=ot[:], in0=ps[:, 0:P], scalar1=rcp[:],
                    scalar2=None, op0=mybir.AluOpType.mult)
                return ot

            for w in range(NW):
                # relation ab -> b rows (out[DSTPAD:])
                o_ab = emit_window_rel("ab", w)
                ob = outp.tile([P, P], BF16, tag="outb")
                nc.scalar.activation(
                    out=ob[:], in_=o_ab[:],
                    func=mybir.ActivationFunctionType.Relu)
                nc.sync.dma_start(
                    out=out[DSTPAD + w * P:DSTPAD + (w + 1) * P, :], in_=ob[:])
                # relations ba, aa -> a rows (out[:DSTPAD])
                o_ba = emit_window_rel("ba", w)
                o_aa = emit_window_rel("aa", w)
                nc.vector.tensor_tensor(
                    out=o_ba[:], in0=o_ba[:], in1=o_aa[:],
                    op=mybir.AluOpType.add)
                oa = outp.tile([P, P], BF16, tag="outa")
                nc.scalar.activation(
                    out=oa[:], in_=o_ba[:],
                    func=mybir.ActivationFunctionType.Relu, scale=0.5)
                nc.sync.dma_start(
                    out=out[w * P:(w + 1) * P, :], in_=oa[:])

    _spill_dma_waits(nc)
    return nc


def _spill_dma_waits(nc):
    """The bundled walrus build only accepts one embedded sync-wait per DMA
    pseudo-instruction. Move multi-waits onto a NoOp on the issuing engine
    (engines decode in order, so the DMA stays gated)."""
    for bbb in nc.bb_map.values():
        insts = bbb.bb.instructions
        out = []
        for ins in insts:
            si = getattr(ins, "sync_info", None)
            ow = list(si.on_wait) if si is not None and si.on_wait else []
            if len(ow) >= 2:
                for w in ow:
                    nop = mybir.InstNoOp(
                        name=nc.get_next_instruction_name(), ins=[], outs=[],
                        engine=ins.engine)
                    nop.sync_info = mybir.SyncInfo(on_wait=[w], on_update=[])
                    out.append(nop)
                ins.sync_info = mybir.SyncInfo(
                    on_wait=[], on_update=list(si.on_update or []))
            out.append(ins)
        insts[:] = out


# ---------------- cached PJRT runner ----------------

_ORIG_RUN_VIA_PJRT = _b2j.run_bass_via_pjrt
_FAST_CACHE = {}


def _fast_run_via_pjrt(nc, in_maps, n_cores):
    if not getattr(nc, "_hgnn_fast", False):
        return _ORIG_RUN_VIA_PJRT(nc, in_maps, n_cores)
    ent = _FAST_CACHE.get(id(nc))
    if ent is None:
        _b2j.install_neuronx_cc_hook()
        partition_name = (nc.partition_id_tensor.name
                          if nc.partition_id_tensor else None)
        in_names, out_names, out_avals = [], [], []
        for alloc in nc.m.functions[0].allocations:
            if not isinstance(alloc, mybir.MemoryLocationSet):
                continue
            name = alloc.memorylocations[0].name
            if alloc.kind == "ExternalInput":
                if name != partition_name:
                    in_names.append(name)
            elif alloc.kind == "ExternalOutput":
                out_names.append(name)
                out_avals.append(jax.core.ShapedArray(
                    tuple(alloc.tensor_shape), mybir.dt.np(alloc.dtype)))
        n_params = len(in_names)
        all_names = list(in_names) + list(out_names)
        if partition_name is not None:
            all_names.append(partition_name)
        all_names = tuple(all_names)

        def _body(*args):
            operands = list(args)
            if partition_name is not None:
                operands.append(_b2j.partition_id_tensor())
            outs = _b2j._bass_exec_p.bind(
                *operands, out_avals=tuple(out_avals), in_names=all_names,
                out_names=tuple(out_names),
                lowering_input_output_aliases=(),
                sim_require_finite=True, sim_require_nnan=True, nc=nc)
            return tuple(outs)

        devices = jax.devices()[:n_cores]
        mesh = Mesh(np.asarray(devices), ("core",))
        nspec = n_params + len(out_names)
        sharded = jax.jit(
            shard_map(_body, mesh=mesh,
                      in_specs=(PartitionSpec("core"),) * nspec,
                      out_specs=(PartitionSpec("core"),) * len(out_names),
                      check_rep=False),
            keep_unused=True)
        sh = NamedSharding(mesh, PartitionSpec("core"))
        zeros = [jax.device_put(
            np.zeros((n_cores * a.shape[0], *a.shape[1:]), a.dtype), sh)
            for a in out_avals]
        jax.block_until_ready(zeros)
        ent = (nc, in_names, out_names, out_avals, sharded, zeros)
        _FAST_CACHE[id(nc)] = ent
    _, in_names, out_names, out_avals, sharded, zeros = ent
    concat_in = [
        np.concatenate([m[nm] for m in in_maps], axis=0) for nm in in_names]
    out_arrs = sharded(*concat_in, *zeros)
    outs_np = [np.asarray(o) for o in out_arrs]
    return [
        {nm: outs_np[i].reshape(n_cores, *out_avals[i].shape)[c]
         for i, nm in enumerate(out_names)}
        for c in range(n_cores)
    ]


_b2j.run_bass_via_pjrt = _fast_run_via_pjrt


# ---------------- host-side preprocessing ----------------

def _balanced_windows(deg):
    """Assign ND dsts to NW bins of <=128 slots, balancing total degree.
    Serpentine assignment over degree-sorted dsts. Returns win[d], slot[d]."""
    order = np.argsort(-deg, kind="stable")
    win = np.empty(ND, np.int32)
    slot = np.empty(ND, np.int32)
    fill = np.zeros(NW, np.int32)
    b = 0
    direction = 1
    for i, d in enumerate(order):
        tries = 0
        while fill[b] >= P:
            b += direction
            if b == NW or b < 0:
                direction = -direction
                b += direction
            tries += 1
            assert tries <= 2 * NW
        win[d] = b
        slot[d] = fill[b]
        fill[b] += 1
        b += direction
        if b == NW or b < 0:
            direction = -direction
            b += direction
    return win, slot


def _pack_edges(src_pos, dst_local, win, slot, sub):
    """Group edges by dst window, pad each window to sub*128 slots.
    src_pos: remapped src indices (uint16 range). Returns srcT uint16 and
    relT uint8 transposed [128, NW*sub] arrays (pad slot rel = 255)."""
    ewin = win[dst_local]
    order = np.argsort(ewin, kind="stable")
    src_s = src_pos[order]
    erel_s = slot[dst_local][order]
    ewin_s = ewin[order]
    counts = np.bincount(ewin_s, minlength=NW)
    offs = np.zeros(NW + 1, np.int64)
    np.cumsum(counts, out=offs[1:])
    pos = np.arange(len(src_s)) - offs[ewin_s]
    flat = ewin_s.astype(np.int64) * (sub * P) + pos
    nslots = NW * sub * P
    srcp = np.zeros(nslots, np.uint16)
    relp = np.full(nslots, 255, np.uint8)
    srcp[flat] = src_s
    relp[flat] = erel_s
    to_T = lambda a: np.ascontiguousarray(a.reshape(NW * sub, P).T)
    return to_T(srcp), to_T(relp)


def kernel(**inputs):
    x_a = np.asarray(inputs["x_a"], np.float32)
    x_b = np.asarray(inputs["x_b"], np.float32)
    edges = {r: np.asarray(inputs[f"edge_{r}"]).astype(np.int64) for r in RELS}

    # packed weights (shared across cores)
    wlr = np.zeros((P, 3 * (HLW + HRW)), np.float32)
    attr = np.zeros((1, 3 * P), np.float32)
    for i, r in enumerate(RELS):
        Wl = np.asarray(inputs[f"Wl_{r}"], np.float32)
        Wr = np.asarray(inputs[f"Wr_{r}"], np.float32)
        att = np.asarray(inputs[f"att_{r}"], np.float32)
        for nm in ("bl", "br", "bias"):
            assert not np.any(np.asarray(inputs[f"{nm}_{r}"])), \
                f"nonzero {nm}_{r} not supported"
        o = i * (HLW + HRW)
        wlr[:, o:o + C] = Wl
        wlr[:, o + 129] = Wl @ att
        wlr[:, o + HLW:o + HLW + C] = Wr
        wlr[:, o + HLW + 128] = Wr @ att
        attr[0, i * P:(i + 1) * P] = att
    wlr16 = wlr.astype(ml_dtypes.bfloat16)

    # sort edges by dst once per relation
    sorted_e = {}
    for r in RELS:
        s, d = edges[r][0], edges[r][1]
        o = np.argsort(d, kind="stable")
        sorted_e[r] = (s[o], d[o])

    # per-core windows (balance combined in-degree per dst type)
    core_loc = []
    winslot = []
    for c in range(NCORES):
        base = c * ND
        deg = {"a": np.zeros(ND, np.int64), "b": np.zeros(ND, np.int64)}
        loc = {}
        for r in RELS:
            s, d = sorted_e[r]
            lo, hi = np.searchsorted(d, [base, base + ND])
            dl = (d[lo:hi] - base).astype(np.int64)
            loc[r] = (s[lo:hi], dl)
            deg[DST_TYPE[r]] += np.bincount(dl, minlength=ND)
        core_loc.append(loc)
        winslot.append({t: _balanced_windows(deg[t]) for t in ("a", "b")})

    # global node -> window-position maps (per type)
    posmap = {}
    for t in ("a", "b"):
        m = np.empty(N, np.int64)
        for c in range(NCORES):
            win, slot = winslot[c][t]
            m[c * ND:(c + 1) * ND] = c * DSTPAD + win.astype(np.int64) * P + slot
        posmap[t] = m

    # global SUB per relation
    subs = {}
    for r in RELS:
        mx = 0
        for c in range(NCORES):
            win, _ = winslot[c][DST_TYPE[r]]
            _, dl = core_loc[c][r]
            wc = np.bincount(win[dl], minlength=NW)
            mx = max(mx, int(wc.max()))
        subs[r] = max(1, -(-mx // P))

    key = tuple(sorted(subs.items()))
    if key not in _BUILD_CACHE:
        _BUILD_CACHE[key] = _build_program(subs)
    nc = _BUILD_CACHE[key]

    in_maps = []
    for c in range(NCORES):
        base = c * ND
        # window-permuted feature slice [128, 12544] bf16
        xl = np.zeros((XW, D), np.float32)
        for t, x in (("a", x_a), ("b", x_b)):
            pos = posmap[t][base:base + ND] - c * DSTPAD
            xl[TYPE_OFF[t] + pos] = x[base:base + ND]
        m = {"xloc": np.ascontiguousarray(xl.T).astype(ml_dtypes.bfloat16),
             "wlr": wlr16, "attr": attr}
        srcs, rels = [], []
        for r in RELS:
            s, dl = core_loc[c][r]
            win, slot = winslot[c][DST_TYPE[r]]
            src_pos = posmap[SRC_TYPE[r]][s]
            sT, rT = _pack_edges(src_pos, dl, win, slot, subs[r])
            srcs.append(sT)
            rels.append(rT)
        m["srcu"] = np.ascontiguousarray(np.concatenate(srcs, axis=1))
        m["relu8"] = np.ascontiguousarray(np.concatenate(rels, axis=1))
        in_maps.append(m)

    res = run_bass_kernel_spmd(nc, in_maps, core_ids=list(range(NCORES)))

    out_a = np.empty((N, C), np.float32)
    out_b = np.empty((N, C), np.float32)
    for c in range(NCORES):
        base = c * ND
        dev = np.asarray(res.results[c]["out"], dtype=np.float32)
        for t, full in (("a", out_a), ("b", out_b)):
            pos = posmap[t][base:base + ND] - c * DSTPAD
            full[base:base + ND] = dev[TYPE_OFF[t] + pos]
    return out_a, out_b
